# revision 18
# baseline (speedup 1.0000x reference)
"""ALiBi sliding-window causal attention (B=2, N=2048, C=1024, H=16, D=64,
W=256) on 8 TRN2 NeuronCores.

Sharding: core = (batch b, sequence chunk c) over a 2x4 grid. Each core owns
512 queries and recomputes K/V for a 256-row halo, so the sliding-window
attention is fully local — no collectives. Matmuls run in bf16 with f32
accumulation; weights/x are pre-transposed and cast on the host.

Key trick: in the S^T = K·Q^T layout (keys on partitions), the ALiBi bias
slope_h*(j - i) splits into a per-key term and a per-query term that cancels
in the normalization. The per-key term slope*(128*jblk + kl - 320) further
splits into a j-independent per-partition part slope*(kl - 64) (fused into a
SINGLE ScalarE exp per (tile, head) covering all key blocks) and a per-
(head, jblk) scalar exp(128*slope*(jblk - 2)) folded into per-head
multiplicative mask tiles (which also carry the window/causal {0,1}
pattern). Head 0's block scalar underflows bf16, so it keeps per-block exps
with the bias in the ScalarE bias operand. The softmax denominator comes
from a ones-column appended to V; core 0's out-of-range halo blocks are
killed by zeroing its halo V rows and ones entries (removing dead keys from
both numerator and denominator).
"""

import contextlib
import math

import numpy as np
import ml_dtypes

import concourse.bass as bass
import concourse.mybir as mybir
import concourse.tile as tile
from concourse.bass_utils import run_bass_kernel_spmd
from concourse.masks import make_identity
from concourse.vector_clock import ScopedClock

# ---------------------------------------------------------------------------
# Patch TileContext._drain_and_barrier: this container's walrus rejects >2 sem
# waits on a CTRL-class instruction ("Too many sync wait commands"), and the
# Tile kernel-tail drain aggregates one wait per live proc. Split the waits
# onto single-wait nop carriers that run just before the drain's barrier.
# ---------------------------------------------------------------------------
_MAX_DRAIN_WAITS = 1


def _patched_drain_and_barrier(self, tick_clock, wait_clock):
    nc = self.nc
    drain_inst = nc.sync.drain()
    wait_clock.add_sem_waits(
        drain_inst.ins, ScopedClock({None: tick_clock.global_clock})
    )
    si = drain_inst.ins.sync_info
    waits = list(si.on_wait) if (si is not None and si.on_wait) else []
    if len(waits) > _MAX_DRAIN_WAITS:
        ups = list(si.on_update) if (si is not None and si.on_update) else []
        drain_inst.ins.sync_info = mybir.SyncInfo(
            on_wait=waits[:_MAX_DRAIN_WAITS], on_update=ups
        )
        for i in range(_MAX_DRAIN_WAITS, len(waits), _MAX_DRAIN_WAITS):
            nop = nc.sync.nop(nofuse=True)
            nop.ins.sync_info = mybir.SyncInfo(
                on_wait=waits[i : i + _MAX_DRAIN_WAITS], on_update=[]
            )

    nc.all_engine_barrier()
    assert self.sems is not None
    popped = nc._tile_sem_poison_stack.pop()
    assert popped is self._sem_poison
    nc.clear_and_free_semaphores(list(self.sems.allocated().values()))


tile.TileContext._drain_and_barrier = _patched_drain_and_barrier

def _dedup_ldweights(nc: bass.Bass):
    """Tile's legalize emits one InstLdweights per matmul even when
    consecutive matmuls use the identical stationary operand. Each load costs
    ~107ns of serial PE time; drop exact-duplicate back-to-back loads (the PE
    array still holds the weights), folding any waits into the next matmul."""
    pe = mybir.EngineType.PE
    for f in nc.m.functions:
        for blk in f.blocks:
            insts = list(blk.instructions)
            new = []
            last_key = None
            pending_waits = []
            changed = False
            for inst in insts:
                if inst.engine != pe:
                    new.append(inst)
                    continue
                tn = type(inst).__name__
                if tn == "InstLdweights":
                    key = (
                        str(inst.ins[0]),
                        str(inst.tile_position),
                        str(inst.tile_size),
                        str(inst.is_transpose),
                        str(inst.perf_mode),
                    )
                    if key == last_key:
                        changed = True
                        si = inst.sync_info
                        if si is not None and si.on_wait:
                            pending_waits.extend(si.on_wait)
                        continue
                    last_key = key
                elif tn != "InstMatmult":
                    pass  # other PE insts don't touch the weight array
                if pending_waits:
                    si = inst.sync_info
                    waits = list(si.on_wait) if (si and si.on_wait) else []
                    ups = list(si.on_update) if (si and si.on_update) else []
                    inst.sync_info = mybir.SyncInfo(
                        on_wait=pending_waits + waits, on_update=ups
                    )
                    pending_waits = []
                new.append(inst)
            if changed:
                blk.instructions = new


_MAX_INST_WAITS = 1


def _split_excess_waits(nc: bass.Bass, max_waits: int = _MAX_INST_WAITS):
    """Walrus in this container rejects instructions carrying more than a
    couple of sem waits. Hoist excess waits onto same-engine nop carriers
    placed immediately before the offending instruction."""
    for f in nc.m.functions:
        for blk in f.blocks:
            snapshot = list(blk.instructions)
            new: list = []
            changed = False
            for inst in snapshot:
                si = inst.sync_info
                waits = list(si.on_wait) if (si is not None and si.on_wait) else []
                if len(waits) > max_waits:
                    changed = True
                    eng = nc.engines[inst.engine]
                    n_extra = len(waits) - max_waits
                    for i in range(0, n_extra, max_waits):
                        chunk = waits[i : min(i + max_waits, n_extra)]
                        nop = eng.nop(nofuse=True)
                        # eng.nop appended to the current bb; reclaim it
                        cur = nc.cur_bb.bb
                        cur.instructions = cur.instructions[:-1]
                        nop.ins.sync_info = mybir.SyncInfo(
                            on_wait=chunk, on_update=[]
                        )
                        new.append(nop.ins)
                    ups = list(si.on_update) if (si is not None and si.on_update) else []
                    inst.sync_info = mybir.SyncInfo(
                        on_wait=waits[n_extra:], on_update=ups
                    )
                new.append(inst)
            if changed:
                blk.instructions = new

# ---------------------------------------------------------------------------
# Problem constants (hardcoded per spec)
# ---------------------------------------------------------------------------
BF16 = ml_dtypes.bfloat16
B, N, C = 2, 2048, 1024
H, D = 16, 64
WINDOW = 256
SCALE = D ** -0.5
NCHUNK = 4  # sequence chunks per batch -> 2*4 = 8 cores
CH = N // NCHUNK  # 512 own rows per core
HALO = WINDOW  # 256 halo rows of K/V context
ROWS = CH + HALO  # 768 rows of x per core
QT_TILES = CH // 128  # 4 query tiles of 128
CBIAS = 64  # per-partition alibi bias centering (overflow/underflow safe)
P = 128
KI = C // P  # 8 contraction tiles
VCOLS = D + 1  # per-head V columns incl. ones column
NCORES = 8


def _alibi_slopes(num_heads: int) -> np.ndarray:
    closest_pow2 = 2 ** math.floor(math.log2(num_heads))
    base = 2.0 ** (-(2.0 ** (-(math.log2(closest_pow2) - 3))))
    powers = np.arange(1, closest_pow2 + 1, dtype=np.float32)
    slopes = base ** powers
    if num_heads != closest_pow2:
        start = 2.0 ** (-(2.0 ** (-(math.log2(closest_pow2) - 3)) - 1))
        extra = np.linspace(start, base, num_heads - closest_pow2, dtype=np.float32)
        slopes = np.concatenate([slopes, extra])
    return slopes.astype(np.float32)


# ---------------------------------------------------------------------------
# Device program
# ---------------------------------------------------------------------------
def build_nc() -> bass.Bass:
    nc = bass.Bass()
    f32 = mybir.dt.float32
    bf16 = mybir.dt.bfloat16

    xt = nc.declare_dram_parameter("xt", [C, ROWS], bf16, isOutput=False)
    wv = nc.declare_dram_parameter("wv", [C, C], bf16, isOutput=False)
    # Q/K weights packed ct-major on the host: wq[ct][p][ki*128+j] =
    # Wq^T[ki*128+p, ct*128+j], so one contiguous DMA delivers everything
    # c_out tile ct needs and the Q_ct/K_ct projections can interleave with
    # the attention loop.
    wq = nc.declare_dram_parameter("wq", [KI, P, C], bf16, isOutput=False)
    wk = nc.declare_dram_parameter("wk", [KI, P, C], bf16, isOutput=False)
    pwt = nc.declare_dram_parameter("pwt", [C, C], bf16, isOutput=False)
    vb = nc.declare_dram_parameter("vb", [C], f32, isOutput=False)
    pb = nc.declare_dram_parameter("pb", [C], f32, isOutput=False)
    # misc pack (contiguous, one DMA): cols 0-7 Q bias per ct, 8-15 K bias
    # per ct, 16-31 per-head batched-exp bias slope_h*(p-64) (col 16 unused),
    # 32-33 head-0 per-block biases (j=1,2), 34-35 core-0 halo kill for V
    # row-blocks 0-1.
    misc = nc.declare_dram_parameter("misc", [P, 36], f32, isOutput=False)
    # per-head multiplicative masks [p(key), h, jblk, q]: window/causal {0,1}
    # pattern times the per-(h, jblk) scalar exp(128*slope_h*(jblk-2)).
    maskh = nc.declare_dram_parameter("maskh", [P, H * 3 * P], bf16, isOutput=False)
    out = nc.declare_dram_parameter("out", [CH, C], f32, isOutput=True)

    with tile.TileContext(nc) as tc, contextlib.ExitStack() as ctx:
        consts = ctx.enter_context(tc.tile_pool(name="consts", bufs=1))
        work = ctx.enter_context(tc.tile_pool(name="work", bufs=6))
        rspool = ctx.enter_context(tc.tile_pool(name="rs", bufs=6))
        finals = ctx.enter_context(tc.tile_pool(name="finals", bufs=2))
        # one dynamic PSUM pool: every tile fits one 2KB bank, 8 banks total
        psum = ctx.enter_context(tc.tile_pool(name="psum", bufs=8, space="PSUM"))

        # ------------------------------- constant loads -------------------
        xt_sb = consts.tile([P, KI, ROWS], bf16, tag="xt")
        wv_sb = consts.tile([P, KI, C], bf16, tag="wv")
        wq_sb = consts.tile([P, KI, C], bf16, tag="wq")  # [p, ct, ki*128]
        wk_sb = consts.tile([P, KI, C], bf16, tag="wk")  # [p, ct, ki*128]
        xt_t = [xt_sb[:, ki, :] for ki in range(KI)]
        wtv_t = [wv_sb[:, ki, :] for ki in range(KI)]
        pwt_sb = consts.tile([P, KI, C], bf16, tag="pwt")
        misc_sb = consts.tile([P, 36], f32, tag="misc")
        vb_sb = consts.tile([P, C], f32, tag="vb")
        pb_sb = consts.tile([P, C], f32, tag="pb")
        maskh_sb = consts.tile([P, H, 3, P], bf16, tag="maskh")
        ident = consts.tile([P, P], bf16, tag="ident")

        xt_r = xt.rearrange("(ko p) n -> p ko n", p=P)
        wv_r = wv.rearrange("(ko p) c -> p ko c", p=P)
        pwt_r = pwt.rearrange("(ko p) c -> p ko c", p=P)
        # DMA order = consumption order: V weights + x first (the V
        # projection is the first compute phase), then per-ct Q/K weights
        # (each Q_ct/K_ct projection is interleaved with the attention
        # iterations of query tile 0), attention constants, proj weights.
        nc.sync.dma_start(out=misc_sb[:], in_=misc[:, :])
        for ki in range(KI):
            nc.sync.dma_start(out=wtv_t[ki], in_=wv_r[:, ki, :])
            nc.sync.dma_start(out=xt_t[ki], in_=xt_r[:, ki, :])
        nc.sync.dma_start(out=vb_sb[:], in_=vb[None, :].to_broadcast((P, C)))
        for ct in range(KI):
            nc.sync.dma_start(out=wq_sb[:, ct, :], in_=wq[ct, :, :])
            nc.sync.dma_start(out=wk_sb[:, ct, :], in_=wk[ct, :, :])
        nc.sync.dma_start(
            out=maskh_sb[:], in_=maskh.rearrange("p (h j q) -> p h j q", h=H, j=3)
        )
        nc.sync.dma_start(out=pb_sb[:], in_=pb[None, :].to_broadcast((P, C)))
        for ki in range(KI):
            nc.sync.dma_start(out=pwt_sb[:, ki, :], in_=pwt_r[:, ki, :])
        make_identity(nc, ident)
        # pre-warm the ScalarE Exp table (~1.3us ACT_TABLE_LOAD) off the
        # attention critical path
        warm = work.tile([P, 1], mybir.dt.float32, tag="warm")
        nc.scalar.activation(
            warm[:], misc_sb[:, 0:1], func=mybir.ActivationFunctionType.Exp
        )

        # ------------------------------- QKV projections ------------------
        # Q^T [c_out, 512 own rows] and K^T [c_out, 768 rows]: c_out on
        # partitions (lhsT = W^T tile), rows on free dim.
        qt_sb = consts.tile([P, KI, CH], bf16, tag="qt")
        kt_sb = consts.tile([P, KI, ROWS], bf16, tag="kt")
        v_sb = consts.tile([P, ROWS // P, H * VCOLS], bf16, tag="v")

        for hcol in range(H):
            nc.vector.memset(v_sb[:, :, hcol * VCOLS + D : hcol * VCOLS + D + 1], 1.0)
        v_view = v_sb.rearrange("p r (h c) -> p r h c", c=VCOLS)

        def emit_v_rb(rb):
            # both c_v chunks inside the ki loop: adjacent matmuls share the
            # stationary x^T tile (one LDWEIGHTS after dedup)
            vps = [
                psum.tile([P, CH], mybir.dt.float32, tag="ps", name=f"vps{_i}")
                for _i in range(2)
            ]
            for ki in range(KI):
                for cc in range(2):
                    nc.tensor.matmul(
                        vps[cc][:],
                        xt_t[ki][:, rb * P : (rb + 1) * P],
                        wtv_t[ki][:, cc * 512 : (cc + 1) * 512],
                        start=(ki == 0),
                        stop=(ki == KI - 1),
                    )
            for cc in range(2):
                nc.vector.tensor_tensor(
                    v_view[:, rb, cc * 8 : (cc + 1) * 8, 0:D],
                    vps[cc][:].rearrange("p (h c) -> p h c", c=D),
                    vb_sb[:, cc * 512 : (cc + 1) * 512].rearrange(
                        "p (h c) -> p h c", c=D
                    ),
                    mybir.AluOpType.add,
                )
            if rb == 1:
                # core 0: zero halo V rows (incl. the ones column) so dead
                # keys vanish from both the PV numerator and the softmax
                # denominator. Other cores multiply by 1.
                nc.vector.tensor_tensor(
                    v_sb[:, 0:2, :],
                    v_sb[:, 0:2, :],
                    misc_sb[:, 34:36, None].to_broadcast((P, 2, H * VCOLS)),
                    mybir.AluOpType.mult,
                )

        def emit_q(ct):
            ps = psum.tile([P, CH], mybir.dt.float32, tag="ps")
            for ki in range(KI):
                nc.tensor.matmul(
                    ps[:],
                    wq_sb[:, ct, ki * P : (ki + 1) * P],
                    xt_t[ki][:, HALO:ROWS],
                    start=(ki == 0),
                    stop=(ki == KI - 1),
                )
            nc.vector.tensor_scalar_add(qt_sb[:, ct, :], ps[:], misc_sb[:, ct : ct + 1])

        def emit_k(ct):
            # both row chunks inside the ki loop: adjacent matmuls share the
            # stationary W tile (one LDWEIGHTS after dedup)
            ps0 = psum.tile([P, CH], mybir.dt.float32, tag="ps")
            ps1 = psum.tile([P, CH], mybir.dt.float32, tag="ps")
            for ki in range(KI):
                w_ap = wk_sb[:, ct, ki * P : (ki + 1) * P]
                nc.tensor.matmul(
                    ps0[:],
                    w_ap,
                    xt_t[ki][:, 0:512],
                    start=(ki == 0),
                    stop=(ki == KI - 1),
                )
                nc.tensor.matmul(
                    ps1[:, :256],
                    w_ap,
                    xt_t[ki][:, 512:ROWS],
                    start=(ki == 0),
                    stop=(ki == KI - 1),
                )
            nc.vector.tensor_scalar_add(
                kt_sb[:, ct, 0:512], ps0[:], misc_sb[:, KI + ct : KI + ct + 1]
            )
            nc.vector.tensor_scalar_add(
                kt_sb[:, ct, 512:ROWS], ps1[:, :256], misc_sb[:, KI + ct : KI + ct + 1]
            )

        # ------------------------------- attention + proj -----------------
        # Flat software-pipelined loop over (t, head-pair): iteration i emits
        # the S^T matmuls + exp of pair i, then the mask-mult / PV matmuls /
        # normalize of pair i-1. This keeps each engine's static FIFO free of
        # head-of-line blocking: when the PE reaches PV(i-1), its pt operand
        # was produced while the PE ran ST(i).
        attn_tiles = {}

        def emit_stage_a(t, hp):
            # heads 0-7 (slopes >= 0.0625): the j=0 key block's ALiBi decay
            # is <= exp(-16) relative to each query's dominant key -
            # numerically invisible next to bf16 noise (measured rel err
            # unchanged to 4 digits), so skip its S^T/exp/PV work entirely
            j_list = (1, 2) if hp <= 3 else (0, 1, 2)
            # the two heads' S^T matmuls contract on disjoint PE row-groups
            # (partitions 0-63 / 64-127); interleaving lets the PE pull each
            # LDWEIGHTS ahead of the in-flight matmul of the other head.
            sts = [
                psum.tile([P, 3, P], mybir.dt.float32, tag="ps", name=f"sts{_i}")
                for _i in range(2)
            ]
            for j in j_list:
                for hi in range(2):
                    po = hi * 64
                    nc.tensor.matmul(
                        sts[hi][:, j, :],
                        kt_sb[po : po + 64, hp, (t + j) * P : (t + j + 1) * P],
                        qt_sb[po : po + 64, hp, t * P : (t + 1) * P],
                        start=True,
                        stop=True,
                    )
            outs = []
            jlo = j_list[0]
            for hi in range(2):
                h = 2 * hp + hi
                exp_t = work.tile([P, 3, P], bf16, tag="exp", name="exp")
                pt = work.tile([P, 3, P], bf16, tag="pt", name="pt")
                if h == 0:
                    # head 0's per-block scalar exp(-128*slope) underflows
                    # bf16: keep per-block exps with the block bias in the
                    # ScalarE bias operand (its mask tile carries plain
                    # {0,1}).
                    for ji, j in enumerate(j_list):
                        nc.scalar.activation(
                            exp_t[:, j, :],
                            sts[hi][:, j, :],
                            func=mybir.ActivationFunctionType.Exp,
                            bias=misc_sb[:, 32 + ji : 33 + ji],
                            scale=1.0,
                        )
                else:
                    # one exp over all live key blocks: bias slope_h*(p-64)
                    # is block-independent; the per-block scalar rides in
                    # the mask multiply of stage_b.
                    nc.scalar.activation(
                        exp_t[:, jlo:3, :],
                        sts[hi][:, jlo:3, :],
                        func=mybir.ActivationFunctionType.Exp,
                        bias=misc_sb[:, 16 + h : 17 + h],
                        scale=1.0,
                    )
                outs.append((exp_t, pt, j_list))
            return outs

        def emit_stage_b(t, hp, work_tiles):
            attn_t = attn_tiles[t]
            o2 = psum.tile([P, 2, VCOLS], mybir.dt.float32, tag="ps", name="o2")
            for hi in range(2):
                h = 2 * hp + hi
                exp_t, pt, j_list = work_tiles[hi]
                jlo = j_list[0]
                nc.vector.tensor_tensor(
                    pt[:, jlo:3, :],
                    exp_t[:, jlo:3, :],
                    maskh_sb[:, h, jlo:3, :],
                    mybir.AluOpType.mult,
                )
                for j in j_list:
                    nc.tensor.matmul(
                        o2[:, hi, :],
                        pt[:, j, :],
                        v_sb[:, t + j, h * VCOLS : (h + 1) * VCOLS],
                        start=(j == j_list[0]),
                        stop=(j == j_list[-1]),
                    )
            rs = rspool.tile([P, 2], mybir.dt.float32, tag="rs", name="rs")
            nc.vector.reciprocal(rs[:], o2[:, :, D])
            nc.vector.tensor_tensor(
                attn_t[:, 2 * hp * D : (2 * hp + 2) * D].rearrange(
                    "p (h d) -> p h d", d=D
                ),
                o2[:, :, 0:D],
                rs[:, :, None].to_broadcast((P, 2, D)),
                mybir.AluOpType.mult,
            )

        at_tiles = {}

        def emit_transpose(t, ct):
            # transpose attn [q, c] -> attnT [c, q] for the output
            # projection; pair hp=ct's normalize wrote exactly these columns
            attn_t = attn_tiles[t]
            at_t = at_tiles[t]
            tr_ps = psum.tile([P, P], bf16, tag="ps", name="tr_ps")
            nc.tensor.transpose(
                tr_ps[:], attn_t[:, ct * P : (ct + 1) * P], ident[:]
            )
            # ScalarE does the PSUM->SBUF move: DVE is the busier engine in
            # the steady state, and ScalarE reads PSUM at full rate
            nc.scalar.copy(at_t[:, ct, :], tr_ps[:])

        def alloc_pps():
            return [
                psum.tile([P, CH], mybir.dt.float32, tag="ps", name=f"pps{_i}")
                for _i in range(2)
            ]

        def emit_proj_pair(t, ct, pps):
            # adjacent matmuls share the stationary attnT tile (one
            # LDWEIGHTS after dedup)
            at_t = at_tiles[t]
            for cc in range(2):
                nc.tensor.matmul(
                    pps[cc][:],
                    at_t[:, ct, :],
                    pwt_sb[:, ct, cc * 512 : (cc + 1) * 512],
                    start=(ct == 0),
                    stop=(ct == KI - 1),
                )

        def emit_fin(t, pps):
            fin = finals.tile([P, C], mybir.dt.float32, tag="fin", name="fin")
            for cc in range(2):
                nc.vector.tensor_tensor(
                    fin[:, cc * 512 : (cc + 1) * 512],
                    pps[cc][:],
                    pb_sb[:, cc * 512 : (cc + 1) * 512],
                    mybir.AluOpType.add,
                )
                # per-half DMA: the first half's writeback overlaps the
                # second half's bias add
                nc.sync.dma_start(
                    out=out[t * P : (t + 1) * P, cc * 512 : (cc + 1) * 512],
                    in_=fin[:, cc * 512 : (cc + 1) * 512],
                )

        def emit_tail(t):
            pps = [
                psum.tile([P, CH], mybir.dt.float32, tag="ps", name=f"pps{_i}")
                for _i in range(2)
            ]
            for ct in range(KI):
                emit_proj_pair(t, ct, pps)
            emit_fin(t, pps)

        HPAIRS = H // 2
        seq = [(t, hp) for t in range(QT_TILES) for hp in range(HPAIRS)]
        DEPTH = 2  # stage_a -> stage_b deferral (iterations)
        TR_EXTRA = 1  # additional deferral of the transpose past stage_b:
        # when the PE reaches transpose(i-3), its normalize (DVE) ran during
        # iteration i-1, so the LDWEIGHTS-transpose never stalls on the
        # reciprocal/normalize chain. proj_pair sits between stage_b and the
        # transpose for the same reason.
        pending = []  # [(t, hp, work_tiles), ...] awaiting stage_b
        tr_pending = []  # [(t, hp), ...] awaiting transpose
        pps_map = {}
        # prologue: just enough V/Q/K so iteration (0,0) can run; the rest
        # of the projections are spliced between attention iterations so
        # ScalarE/DVE attention work overlaps the PE-bound projections.
        emit_v_rb(0)
        emit_v_rb(1)
        emit_q(0)
        emit_k(0)
        emit_v_rb(2)
        emit_q(1)
        emit_k(1)
        for i, (t, hp) in enumerate(seq):
            if t == 0 and 1 <= hp <= 6:
                # Q/K chunk hp+1 one iteration ahead of its first use
                emit_q(hp + 1)
                emit_k(hp + 1)
            if i == 8:
                emit_v_rb(3)  # first needed by stage_b(1,0) at i=10
            elif i == 12:
                emit_v_rb(4)  # first needed by stage_b(2,0) at i=18
            elif i == 20:
                emit_v_rb(5)  # first needed by stage_b(3,0) at i=26
            if hp == 0:
                attn_tiles[t] = consts.tile(
                    [P, C], bf16, tag=f"attn_{t}", name=f"attn_{t}"
                )
                at_tiles[t] = consts.tile(
                    [P, KI, P], bf16, tag=f"attnT_{t}", name=f"at_{t}"
                )
                if t > 0:
                    pps_map[t - 1] = alloc_pps()
            wts_ = emit_stage_a(t, hp)
            if len(pending) >= DEPTH:
                pt_, php, pwts = pending.pop(0)
                emit_stage_b(pt_, php, pwts)
                tr_pending.append((pt_, php))
            if t > 0:
                # spread the previous tile's output projection across this
                # tile's iterations: keeps the PE continuously fed (HAM warm)
                # instead of a cold burst at each t boundary
                emit_proj_pair(t - 1, hp, pps_map[t - 1])
                if hp == HPAIRS - 1:
                    emit_fin(t - 1, pps_map.pop(t - 1))
            if len(tr_pending) > TR_EXTRA:
                emit_transpose(*tr_pending.pop(0))
            pending.append((t, hp, wts_))
        t_last = seq[-1][0]
        pps_last = alloc_pps()
        for ct in range(HPAIRS - DEPTH - TR_EXTRA):
            emit_proj_pair(t_last, ct, pps_last)
        while pending:
            pt_, php, pwts = pending.pop(0)
            emit_stage_b(pt_, php, pwts)
            tr_pending.append((pt_, php))
            trt, trp = tr_pending.pop(0)
            emit_transpose(trt, trp)
            emit_proj_pair(trt, trp, pps_last)
        while tr_pending:
            trt, trp = tr_pending.pop(0)
            emit_transpose(trt, trp)
            emit_proj_pair(trt, trp, pps_last)
        emit_fin(t_last, pps_last)

    _dedup_ldweights(nc)
    _split_excess_waits(nc)
    return nc


_NC_CACHE = None


def _get_nc() -> bass.Bass:
    global _NC_CACHE
    if _NC_CACHE is None:
        _NC_CACHE = build_nc()
    return _NC_CACHE


# ---------------------------------------------------------------------------
# Host side: shard, pre-transpose, cast; run SPMD; gather
# ---------------------------------------------------------------------------
def make_in_maps(x, qkv_w, qkv_b, proj_w, proj_b):
    x = np.asarray(x, np.float32)
    qkv_w = np.asarray(qkv_w, np.float32)
    qkv_b = np.asarray(qkv_b, np.float32)
    proj_w = np.asarray(proj_w, np.float32)
    proj_b = np.asarray(proj_b, np.float32)

    # fold the attention scale into the Q projection
    qkv_w = qkv_w.copy()
    qkv_b = qkv_b.copy()
    qkv_w[:C] *= SCALE
    qkv_b[:C] *= SCALE

    wv_np = np.ascontiguousarray(qkv_w[2 * C :].T).astype(BF16)

    def _ct_major(w):  # [c_out, c_in] -> [ct, p, ki*128]
        wt_ = w.T  # [c_in, c_out]
        return np.ascontiguousarray(
            wt_.reshape(KI, P, KI, P).transpose(2, 1, 0, 3).reshape(KI, P, C)
        ).astype(BF16)

    wq_np = _ct_major(qkv_w[:C])
    wk_np = _ct_major(qkv_w[C : 2 * C])
    pwt_np = np.ascontiguousarray(proj_w.T).astype(BF16)
    vb_np = np.ascontiguousarray(qkv_b[2 * C :])
    pb_np = proj_b

    slopes = _alibi_slopes(H)
    pp = np.arange(P, dtype=np.float32)

    # misc pack [P, 36] f32 (see device decl for the column map)
    misc_np = np.zeros((P, 36), np.float32)
    misc_np[:, 0:16] = qkv_b[: 2 * C].reshape(16, P).T
    misc_np[:, 16:32] = slopes[None, :] * (pp[:, None] - CBIAS)
    misc_np[:, 16] = 0.0
    for ji, j in enumerate((1, 2)):
        misc_np[:, 32 + ji] = slopes[0] * (j * P + pp - 320.0)
    misc_np[:, 34:36] = 1.0
    misc0_np = misc_np.copy()
    misc0_np[:, 34:36] = 0.0  # core 0: halo V rows 0-255 killed

    # per-head mask tiles [P(key), H, 3, P(query)]: {0,1} window/causal
    # pattern x per-(h, jblk) scalar exp(128*slope*(jblk-2)); head 0 plain.
    kk = pp[:, None]
    qq = pp[None, :]
    upper = (kk > qq).astype(np.float32)
    lower = (kk <= qq).astype(np.float32)
    ones = np.ones((P, P), np.float32)
    maskh_np = np.zeros((P, H, 3, P), np.float32)
    for h in range(H):
        c0 = math.exp(-256.0 * float(slopes[h]))
        c1 = math.exp(-128.0 * float(slopes[h]))
        if h == 0:
            maskh_np[:, h, 1] = ones
            maskh_np[:, h, 2] = lower
        else:
            if h >= 8:
                maskh_np[:, h, 0] = upper * c0
            maskh_np[:, h, 1] = ones * c1
            maskh_np[:, h, 2] = lower
    maskh_np = np.ascontiguousarray(maskh_np.reshape(P, H * 3 * P)).astype(BF16)

    in_maps = []
    for core in range(NCORES):
        b, c = divmod(core, NCHUNK)
        n0 = c * CH
        xh = np.zeros((ROWS, C), np.float32)
        lo = max(0, n0 - HALO)
        xh[HALO - (n0 - lo) :] = x[b, lo : n0 + CH]
        in_maps.append(
            {
                "xt": np.ascontiguousarray(xh.T).astype(BF16),
                "wv": wv_np,
                "wq": wq_np,
                "wk": wk_np,
                "pwt": pwt_np,
                "vb": vb_np,
                "pb": pb_np,
                "misc": misc0_np if c == 0 else misc_np,
                "maskh": maskh_np,
            }
        )
    return in_maps


def run(in_maps, trace=False, **kw):
    res = run_bass_kernel_spmd(
        _get_nc(), in_maps, core_ids=list(range(NCORES)), trace=trace, **kw
    )
    return res


def assemble(res):
    out = np.empty((B, N, C), np.float32)
    for core in range(NCORES):
        b, c = divmod(core, NCHUNK)
        out[b, c * CH : (c + 1) * CH] = res.results[core]["out"]
    return out


def kernel(x, qkv_w, qkv_b, proj_w, proj_b):
    in_maps = make_in_maps(x, qkv_w, qkv_b, proj_w, proj_b)
    res = run(in_maps)
    return assemble(res)



# revision 37
# speedup vs baseline: 1.0599x; 1.0599x over previous
"""ALiBi sliding-window causal attention (B=2, N=2048, C=1024, H=16, D=64,
W=256) on 8 TRN2 NeuronCores.

Sharding: core = (batch b, sequence chunk c) over a 2x4 grid. Each core owns
512 queries and recomputes K/V for a 256-row halo, so the sliding-window
attention is fully local — no collectives. Matmuls run in bf16 with f32
accumulation; weights/x are pre-transposed and cast on the host.

Key trick: in the S^T = K·Q^T layout (keys on partitions), the ALiBi bias
slope_h*(j - i) splits into a per-key term and a per-query term that cancels
in the normalization. The per-key term slope*(128*jblk + kl - 320) further
splits into a j-independent per-partition part slope*(kl - 64) (fused into a
SINGLE ScalarE exp per (tile, head) covering all key blocks) and a per-
(head, jblk) scalar exp(128*slope*(jblk - 2)) folded into per-head
multiplicative mask tiles (which also carry the window/causal {0,1}
pattern). Head 0's block scalar underflows bf16, so it keeps per-block exps
with the bias in the ScalarE bias operand. The softmax denominator comes
from a ones-column appended to V; core 0's out-of-range halo blocks are
killed by zeroing its halo V rows and ones entries (removing dead keys from
both numerator and denominator).
"""

import contextlib
import math

import numpy as np
import ml_dtypes

import concourse.bass as bass
import concourse.mybir as mybir
import concourse.tile as tile
from concourse.bass_utils import run_bass_kernel_spmd
from concourse.masks import make_identity
from concourse.vector_clock import ScopedClock

# ---------------------------------------------------------------------------
# Patch TileContext._drain_and_barrier: this container's walrus rejects >2 sem
# waits on a CTRL-class instruction ("Too many sync wait commands"), and the
# Tile kernel-tail drain aggregates one wait per live proc. Split the waits
# onto single-wait nop carriers that run just before the drain's barrier.
# ---------------------------------------------------------------------------
_MAX_DRAIN_WAITS = 1


def _patched_drain_and_barrier(self, tick_clock, wait_clock):
    nc = self.nc
    drain_inst = nc.sync.drain()
    wait_clock.add_sem_waits(
        drain_inst.ins, ScopedClock({None: tick_clock.global_clock})
    )
    si = drain_inst.ins.sync_info
    waits = list(si.on_wait) if (si is not None and si.on_wait) else []
    if len(waits) > _MAX_DRAIN_WAITS:
        ups = list(si.on_update) if (si is not None and si.on_update) else []
        drain_inst.ins.sync_info = mybir.SyncInfo(
            on_wait=waits[:_MAX_DRAIN_WAITS], on_update=ups
        )
        for i in range(_MAX_DRAIN_WAITS, len(waits), _MAX_DRAIN_WAITS):
            nop = nc.sync.nop(nofuse=True)
            nop.ins.sync_info = mybir.SyncInfo(
                on_wait=waits[i : i + _MAX_DRAIN_WAITS], on_update=[]
            )

    nc.all_engine_barrier()
    assert self.sems is not None
    popped = nc._tile_sem_poison_stack.pop()
    assert popped is self._sem_poison
    nc.clear_and_free_semaphores(list(self.sems.allocated().values()))


tile.TileContext._drain_and_barrier = _patched_drain_and_barrier

def _dedup_ldweights(nc: bass.Bass):
    """Tile's legalize emits one InstLdweights per matmul even when
    consecutive matmuls use the identical stationary operand. Each load costs
    ~107ns of serial PE time; drop exact-duplicate back-to-back loads (the PE
    array still holds the weights), folding any waits into the next matmul."""
    pe = mybir.EngineType.PE
    for f in nc.m.functions:
        for blk in f.blocks:
            insts = list(blk.instructions)
            new = []
            last_key = None
            pending_waits = []
            changed = False
            for inst in insts:
                if inst.engine != pe:
                    new.append(inst)
                    continue
                tn = type(inst).__name__
                if tn == "InstLdweights":
                    key = (
                        str(inst.ins[0]),
                        str(inst.tile_position),
                        str(inst.tile_size),
                        str(inst.is_transpose),
                        str(inst.perf_mode),
                    )
                    if key == last_key:
                        changed = True
                        si = inst.sync_info
                        if si is not None and si.on_wait:
                            pending_waits.extend(si.on_wait)
                        continue
                    last_key = key
                elif tn != "InstMatmult":
                    pass  # other PE insts don't touch the weight array
                if pending_waits:
                    si = inst.sync_info
                    waits = list(si.on_wait) if (si and si.on_wait) else []
                    ups = list(si.on_update) if (si and si.on_update) else []
                    inst.sync_info = mybir.SyncInfo(
                        on_wait=pending_waits + waits, on_update=ups
                    )
                    pending_waits = []
                new.append(inst)
            if changed:
                blk.instructions = new


_MAX_INST_WAITS = 1


def _split_excess_waits(nc: bass.Bass, max_waits: int = _MAX_INST_WAITS):
    """Walrus in this container rejects instructions carrying more than a
    couple of sem waits. Hoist excess waits onto same-engine nop carriers
    placed immediately before the offending instruction."""
    for f in nc.m.functions:
        for blk in f.blocks:
            snapshot = list(blk.instructions)
            new: list = []
            changed = False
            for inst in snapshot:
                si = inst.sync_info
                waits = list(si.on_wait) if (si is not None and si.on_wait) else []
                if len(waits) > max_waits:
                    changed = True
                    eng = nc.engines[inst.engine]
                    n_extra = len(waits) - max_waits
                    for i in range(0, n_extra, max_waits):
                        chunk = waits[i : min(i + max_waits, n_extra)]
                        nop = eng.nop(nofuse=True)
                        # eng.nop appended to the current bb; reclaim it
                        cur = nc.cur_bb.bb
                        cur.instructions = cur.instructions[:-1]
                        nop.ins.sync_info = mybir.SyncInfo(
                            on_wait=chunk, on_update=[]
                        )
                        new.append(nop.ins)
                    ups = list(si.on_update) if (si is not None and si.on_update) else []
                    inst.sync_info = mybir.SyncInfo(
                        on_wait=waits[n_extra:], on_update=ups
                    )
                new.append(inst)
            if changed:
                blk.instructions = new

# ---------------------------------------------------------------------------
# Problem constants (hardcoded per spec)
# ---------------------------------------------------------------------------
BF16 = ml_dtypes.bfloat16
F8 = ml_dtypes.float8_e4m3  # TRN FP8_EXP4 semantics (bias 7, max 240)
KI2 = 4  # fp8 DoubleRow contraction tiles (256 each)
WQ_UP = 64.0  # host-side Wq upscale (dodges fp8 subnormals), undone on DVE
WK_UP = 8.0
B, N, C = 2, 2048, 1024
H, D = 16, 64
WINDOW = 256
SCALE = D ** -0.5
NCHUNK = 4  # sequence chunks per batch -> 2*4 = 8 cores
CH = N // NCHUNK  # 512 own rows per core
HALO = WINDOW  # 256 halo rows of K/V context
ROWS = CH + HALO  # 768 rows of x per core
QT_TILES = CH // 128  # 4 query tiles of 128
CBIAS = 64  # per-partition alibi bias centering (overflow/underflow safe)
P = 128
KI = C // P  # 8 contraction tiles
VCOLS = D + 1  # per-head V columns incl. ones column
NCORES = 8


def _alibi_slopes(num_heads: int) -> np.ndarray:
    closest_pow2 = 2 ** math.floor(math.log2(num_heads))
    base = 2.0 ** (-(2.0 ** (-(math.log2(closest_pow2) - 3))))
    powers = np.arange(1, closest_pow2 + 1, dtype=np.float32)
    slopes = base ** powers
    if num_heads != closest_pow2:
        start = 2.0 ** (-(2.0 ** (-(math.log2(closest_pow2) - 3)) - 1))
        extra = np.linspace(start, base, num_heads - closest_pow2, dtype=np.float32)
        slopes = np.concatenate([slopes, extra])
    return slopes.astype(np.float32)


# ---------------------------------------------------------------------------
# Device program
# ---------------------------------------------------------------------------
def build_nc() -> bass.Bass:
    nc = bass.Bass()
    f32 = mybir.dt.float32
    bf16 = mybir.dt.bfloat16

    xt = nc.declare_dram_parameter("xt", [C, ROWS], bf16, isOutput=False)
    wv = nc.declare_dram_parameter("wv", [C, C], bf16, isOutput=False)
    # Q/K weights packed ct-major on the host: wq[ct][p][ki*128+j] =
    # Wq^T[ki*128+p, ct*128+j], so one contiguous DMA delivers everything
    # c_out tile ct needs and the Q_ct/K_ct projections can interleave with
    # the attention loop.
    wq = nc.declare_dram_parameter("wq", [KI, P, C], bf16, isOutput=False)
    wk = nc.declare_dram_parameter("wk", [KI, P, C], bf16, isOutput=False)
    pwt = nc.declare_dram_parameter("pwt", [C, C], bf16, isOutput=False)
    vb = nc.declare_dram_parameter("vb", [C], f32, isOutput=False)
    pb = nc.declare_dram_parameter("pb", [C], f32, isOutput=False)
    # misc pack (contiguous, one DMA): cols 0-7 Q bias per ct, 8-15 K bias
    # per ct, 16-31 per-head batched-exp bias slope_h*(p-64) (col 16 unused),
    # 32-33 head-0 per-block biases (j=1,2), 34-35 core-0 halo kill for V
    # row-blocks 0-1.
    misc = nc.declare_dram_parameter("misc", [P, 36], f32, isOutput=False)
    # per-head multiplicative masks [p(key), h, jblk, q]: window/causal {0,1}
    # pattern times the per-(h, jblk) scalar exp(128*slope_h*(jblk-2)).
    maskh = nc.declare_dram_parameter("maskh", [P, H * 3 * P], bf16, isOutput=False)
    out = nc.declare_dram_parameter("out", [CH, C], f32, isOutput=True)

    with tile.TileContext(nc) as tc, contextlib.ExitStack() as ctx:
        consts = ctx.enter_context(tc.tile_pool(name="consts", bufs=1))
        work = ctx.enter_context(tc.tile_pool(name="work", bufs=6))
        rspool = ctx.enter_context(tc.tile_pool(name="rs", bufs=6))
        finals = ctx.enter_context(tc.tile_pool(name="finals", bufs=2))
        # one dynamic PSUM pool: every tile fits one 2KB bank, 8 banks total
        psum = ctx.enter_context(tc.tile_pool(name="psum", bufs=8, space="PSUM"))

        # ------------------------------- constant loads -------------------
        xt_sb = consts.tile([P, KI, ROWS], bf16, tag="xt")
        wv_sb = consts.tile([P, KI, C], bf16, tag="wv")
        wq_sb = consts.tile([P, KI, C], bf16, tag="wq")  # [p, ct, ki*128]
        wk_sb = consts.tile([P, KI, C], bf16, tag="wk")  # [p, ct, ki*128]
        xt_t = [xt_sb[:, ki, :] for ki in range(KI)]
        wtv_t = [wv_sb[:, ki, :] for ki in range(KI)]
        pwt_sb = consts.tile([P, KI, C], bf16, tag="pwt")
        misc_sb = consts.tile([P, 36], f32, tag="misc")
        vb_sb = consts.tile([P, C], f32, tag="vb")
        pb_sb = consts.tile([P, C], f32, tag="pb")
        maskh_sb = consts.tile([P, H, 3, P], bf16, tag="maskh")
        ident = consts.tile([P, P], bf16, tag="ident")

        xt_r = xt.rearrange("(ko p) n -> p ko n", p=P)
        wv_r = wv.rearrange("(ko p) c -> p ko c", p=P)
        pwt_r = pwt.rearrange("(ko p) c -> p ko c", p=P)
        # DMA order = consumption order: V weights + x first (the V
        # projection is the first compute phase), then per-ct Q/K weights
        # (each Q_ct/K_ct projection is interleaved with the attention
        # iterations of query tile 0), attention constants, proj weights.
        nc.sync.dma_start(out=misc_sb[:], in_=misc[:, :])
        for ki in range(KI):
            nc.sync.dma_start(out=wtv_t[ki], in_=wv_r[:, ki, :])
            nc.sync.dma_start(out=xt_t[ki], in_=xt_r[:, ki, :])
        nc.sync.dma_start(out=vb_sb[:], in_=vb[None, :].to_broadcast((P, C)))
        for ct in range(KI):
            nc.sync.dma_start(out=wq_sb[:, ct, :], in_=wq[ct, :, :])
            nc.sync.dma_start(out=wk_sb[:, ct, :], in_=wk[ct, :, :])
        nc.sync.dma_start(
            out=maskh_sb[:], in_=maskh.rearrange("p (h j q) -> p h j q", h=H, j=3)
        )
        nc.sync.dma_start(out=pb_sb[:], in_=pb[None, :].to_broadcast((P, C)))
        for ki in range(KI):
            nc.sync.dma_start(out=pwt_sb[:, ki, :], in_=pwt_r[:, ki, :])
        make_identity(nc, ident)
        # pre-warm the ScalarE Exp table (~1.3us ACT_TABLE_LOAD) off the
        # attention critical path
        warm = work.tile([P, 1], mybir.dt.float32, tag="warm")
        nc.scalar.activation(
            warm[:], misc_sb[:, 0:1], func=mybir.ActivationFunctionType.Exp
        )

        # ------------------------------- QKV projections ------------------
        # Q^T [c_out, 512 own rows] and K^T [c_out, 768 rows]: c_out on
        # partitions (lhsT = W^T tile), rows on free dim.
        qt_sb = consts.tile([P, KI, CH], bf16, tag="qt")
        kt_sb = consts.tile([P, KI, ROWS], bf16, tag="kt")
        v_sb = consts.tile([P, ROWS // P, H * VCOLS], bf16, tag="v")

        for hcol in range(H):
            nc.vector.memset(v_sb[:, :, hcol * VCOLS + D : hcol * VCOLS + D + 1], 1.0)
        v_view = v_sb.rearrange("p r (h c) -> p r h c", c=VCOLS)

        def _v_epilogue(rb, vps):
            for cc in range(2):
                nc.vector.tensor_tensor(
                    v_view[:, rb, cc * 8 : (cc + 1) * 8, 0:D],
                    vps[cc][:].rearrange("p (h c) -> p h c", c=D),
                    vb_sb[:, cc * 512 : (cc + 1) * 512].rearrange(
                        "p (h c) -> p h c", c=D
                    ),
                    mybir.AluOpType.add,
                )
            if rb == 1:
                # core 0: zero halo V rows (incl. the ones column) so dead
                # keys vanish from both the PV numerator and the softmax
                # denominator. Other cores multiply by 1.
                nc.vector.tensor_tensor(
                    v_sb[:, 0:2, :],
                    v_sb[:, 0:2, :],
                    misc_sb[:, 34:36, None].to_broadcast((P, 2, H * VCOLS)),
                    mybir.AluOpType.mult,
                )

        def emit_v_rb(rb):
            # both c_v chunks inside the ki loop: adjacent matmuls share the
            # stationary x^T tile (one LDWEIGHTS after dedup)
            vps = [
                psum.tile([P, CH], mybir.dt.float32, tag="ps", name=f"vps{_i}")
                for _i in range(2)
            ]
            for ki in range(KI):
                for cc in range(2):
                    nc.tensor.matmul(
                        vps[cc][:],
                        xt_t[ki][:, rb * P : (rb + 1) * P],
                        wtv_t[ki][:, cc * 512 : (cc + 1) * 512],
                        start=(ki == 0),
                        stop=(ki == KI - 1),
                    )
            _v_epilogue(rb, vps)

        def emit_v_rb012():
            # prologue V for row blocks 0-2 with ki OUTERMOST (6 psum banks):
            # the per-ki work (6 matmuls, ~1.3us) matches the wtv/xt DMA-pair
            # arrival rate, so the PE tracks the weight stream instead of
            # idling through rb0 and serializing rb1/rb2 after it.
            vps = {
                (rb, cc): psum.tile(
                    [P, CH], mybir.dt.float32, tag="ps", name=f"vp{rb}{cc}"
                )
                for rb in range(3)
                for cc in range(2)
            }
            for ki in range(KI):
                for rb in range(3):
                    for cc in range(2):
                        nc.tensor.matmul(
                            vps[(rb, cc)][:],
                            xt_t[ki][:, rb * P : (rb + 1) * P],
                            wtv_t[ki][:, cc * 512 : (cc + 1) * 512],
                            start=(ki == 0),
                            stop=(ki == KI - 1),
                        )
            for rb in range(3):
                _v_epilogue(rb, [vps[(rb, 0)], vps[(rb, 1)]])

        def emit_q(ct):
            ps = psum.tile([P, CH], mybir.dt.float32, tag="ps")
            for ki in range(KI):
                nc.tensor.matmul(
                    ps[:],
                    wq_sb[:, ct, ki * P : (ki + 1) * P],
                    xt_t[ki][:, HALO:ROWS],
                    start=(ki == 0),
                    stop=(ki == KI - 1),
                )
            nc.vector.tensor_scalar_add(qt_sb[:, ct, :], ps[:], misc_sb[:, ct : ct + 1])

        def emit_k(ct):
            # both row chunks inside the ki loop: adjacent matmuls share the
            # stationary W tile (one LDWEIGHTS after dedup)
            ps0 = psum.tile([P, CH], mybir.dt.float32, tag="ps")
            ps1 = psum.tile([P, CH], mybir.dt.float32, tag="ps")
            for ki in range(KI):
                w_ap = wk_sb[:, ct, ki * P : (ki + 1) * P]
                nc.tensor.matmul(
                    ps0[:],
                    w_ap,
                    xt_t[ki][:, 0:512],
                    start=(ki == 0),
                    stop=(ki == KI - 1),
                )
                nc.tensor.matmul(
                    ps1[:, :256],
                    w_ap,
                    xt_t[ki][:, 512:ROWS],
                    start=(ki == 0),
                    stop=(ki == KI - 1),
                )
            nc.vector.tensor_scalar_add(
                kt_sb[:, ct, 0:512], ps0[:], misc_sb[:, KI + ct : KI + ct + 1]
            )
            nc.vector.tensor_scalar_add(
                kt_sb[:, ct, 512:ROWS], ps1[:, :256], misc_sb[:, KI + ct : KI + ct + 1]
            )

        # ------------------------------- attention + proj -----------------
        # Flat software-pipelined loop over (t, head-pair): iteration i emits
        # the S^T matmuls + exp of pair i, then the mask-mult / PV matmuls /
        # normalize of pair i-1. This keeps each engine's static FIFO free of
        # head-of-line blocking: when the PE reaches PV(i-1), its pt operand
        # was produced while the PE ran ST(i).
        attn_tiles = {}

        def emit_stage_a(t, hp):
            # heads 0-7 (slopes >= 0.0625): the j=0 key block's ALiBi decay
            # is <= exp(-16) relative to each query's dominant key -
            # numerically invisible next to bf16 noise (measured rel err
            # unchanged to 4 digits), so skip its S^T/exp/PV work entirely
            j_list = (1, 2) if hp <= 3 else (0, 1, 2)
            # the two heads' S^T matmuls contract on disjoint PE row-groups
            # (partitions 0-63 / 64-127); interleaving lets the PE pull each
            # LDWEIGHTS ahead of the in-flight matmul of the other head.
            sts = [
                psum.tile([P, 3, P], mybir.dt.float32, tag="ps", name=f"sts{_i}")
                for _i in range(2)
            ]
            for j in j_list:
                for hi in range(2):
                    po = hi * 64
                    nc.tensor.matmul(
                        sts[hi][:, j, :],
                        kt_sb[po : po + 64, hp, (t + j) * P : (t + j + 1) * P],
                        qt_sb[po : po + 64, hp, t * P : (t + 1) * P],
                        start=True,
                        stop=True,
                    )
            outs = []
            jlo = j_list[0]
            for hi in range(2):
                h = 2 * hp + hi
                exp_t = work.tile([P, 3, P], bf16, tag="exp", name="exp")
                pt = work.tile([P, 3, P], bf16, tag="pt", name="pt")
                if h == 0:
                    # head 0's per-block scalar exp(-128*slope) underflows
                    # bf16: keep per-block exps with the block bias in the
                    # ScalarE bias operand (its mask tile carries plain
                    # {0,1}).
                    for ji, j in enumerate(j_list):
                        nc.scalar.activation(
                            exp_t[:, j, :],
                            sts[hi][:, j, :],
                            func=mybir.ActivationFunctionType.Exp,
                            bias=misc_sb[:, 32 + ji : 33 + ji],
                            scale=1.0,
                        )
                else:
                    # one exp over all live key blocks: bias slope_h*(p-64)
                    # is block-independent; the per-block scalar rides in
                    # the mask multiply of stage_b.
                    nc.scalar.activation(
                        exp_t[:, jlo:3, :],
                        sts[hi][:, jlo:3, :],
                        func=mybir.ActivationFunctionType.Exp,
                        bias=misc_sb[:, 16 + h : 17 + h],
                        scale=1.0,
                    )
                outs.append((exp_t, pt, j_list))
            return outs

        def emit_stage_b(t, hp, work_tiles):
            attn_t = attn_tiles[t]
            o2 = psum.tile([P, 2, VCOLS], mybir.dt.float32, tag="ps", name="o2")
            for hi in range(2):
                h = 2 * hp + hi
                exp_t, pt, j_list = work_tiles[hi]
                jlo = j_list[0]
                nc.vector.tensor_tensor(
                    pt[:, jlo:3, :],
                    exp_t[:, jlo:3, :],
                    maskh_sb[:, h, jlo:3, :],
                    mybir.AluOpType.mult,
                )
                for j in j_list:
                    nc.tensor.matmul(
                        o2[:, hi, :],
                        pt[:, j, :],
                        v_sb[:, t + j, h * VCOLS : (h + 1) * VCOLS],
                        start=(j == j_list[0]),
                        stop=(j == j_list[-1]),
                    )
            rs = rspool.tile([P, 2], mybir.dt.float32, tag="rs", name="rs")
            nc.vector.reciprocal(rs[:], o2[:, :, D])
            nc.vector.tensor_tensor(
                attn_t[:, 2 * hp * D : (2 * hp + 2) * D].rearrange(
                    "p (h d) -> p h d", d=D
                ),
                o2[:, :, 0:D],
                rs[:, :, None].to_broadcast((P, 2, D)),
                mybir.AluOpType.mult,
            )

        at_tiles = {}

        def emit_transpose(t, ct):
            # transpose attn [q, c] -> attnT [c, q] for the output
            # projection; pair hp=ct's normalize wrote exactly these columns
            attn_t = attn_tiles[t]
            at_t = at_tiles[t]
            tr_ps = psum.tile([P, P], bf16, tag="ps", name="tr_ps")
            nc.tensor.transpose(
                tr_ps[:], attn_t[:, ct * P : (ct + 1) * P], ident[:]
            )
            # ScalarE does the PSUM->SBUF move: DVE is the busier engine in
            # the steady state, and ScalarE reads PSUM at full rate
            nc.scalar.copy(at_t[:, ct, :], tr_ps[:])

        def alloc_pps():
            return [
                psum.tile([P, CH], mybir.dt.float32, tag="ps", name=f"pps{_i}")
                for _i in range(2)
            ]

        def emit_proj_pair(t, ct, pps):
            # adjacent matmuls share the stationary attnT tile (one
            # LDWEIGHTS after dedup)
            at_t = at_tiles[t]
            for cc in range(2):
                nc.tensor.matmul(
                    pps[cc][:],
                    at_t[:, ct, :],
                    pwt_sb[:, ct, cc * 512 : (cc + 1) * 512],
                    start=(ct == 0),
                    stop=(ct == KI - 1),
                )

        def emit_fin(t, pps):
            fin = finals.tile([P, C], mybir.dt.float32, tag="fin", name="fin")
            for cc in range(2):
                nc.vector.tensor_tensor(
                    fin[:, cc * 512 : (cc + 1) * 512],
                    pps[cc][:],
                    pb_sb[:, cc * 512 : (cc + 1) * 512],
                    mybir.AluOpType.add,
                )
                # per-half DMA: the first half's writeback overlaps the
                # second half's bias add
                nc.sync.dma_start(
                    out=out[t * P : (t + 1) * P, cc * 512 : (cc + 1) * 512],
                    in_=fin[:, cc * 512 : (cc + 1) * 512],
                )

        def emit_tail(t):
            pps = [
                psum.tile([P, CH], mybir.dt.float32, tag="ps", name=f"pps{_i}")
                for _i in range(2)
            ]
            for ct in range(KI):
                emit_proj_pair(t, ct, pps)
            emit_fin(t, pps)

        HPAIRS = H // 2
        seq = [(t, hp) for t in range(QT_TILES) for hp in range(HPAIRS)]
        DEPTH = 2  # stage_a -> stage_b deferral (iterations)
        TR_EXTRA = 1  # additional deferral of the transpose past stage_b:
        # when the PE reaches transpose(i-3), its normalize (DVE) ran during
        # iteration i-1, so the LDWEIGHTS-transpose never stalls on the
        # reciprocal/normalize chain. proj_pair sits between stage_b and the
        # transpose for the same reason.
        pending = []  # [(t, hp, work_tiles), ...] awaiting stage_b
        tr_pending = []  # [(t, hp), ...] awaiting transpose
        pps_map = {}
        # prologue: just enough V/Q/K so iteration (0,0) can run; the rest
        # of the projections are spliced between attention iterations so
        # ScalarE/DVE attention work overlaps the PE-bound projections.
        emit_v_rb012()
        emit_q(0)
        emit_k(0)
        for i, (t, hp) in enumerate(seq):
            if t == 0 and hp <= 6:
                # Q/K chunk hp+1 one iteration ahead of its first use
                emit_q(hp + 1)
                emit_k(hp + 1)
            if i == 8:
                emit_v_rb(3)  # first needed by stage_b(1,0) at i=10
            elif i == 12:
                emit_v_rb(4)  # first needed by stage_b(2,0) at i=18
            elif i == 20:
                emit_v_rb(5)  # first needed by stage_b(3,0) at i=26
            if hp == 0:
                attn_tiles[t] = consts.tile(
                    [P, C], bf16, tag=f"attn_{t}", name=f"attn_{t}"
                )
                at_tiles[t] = consts.tile(
                    [P, KI, P], bf16, tag=f"attnT_{t}", name=f"at_{t}"
                )
                if t > 0:
                    pps_map[t - 1] = alloc_pps()
            wts_ = emit_stage_a(t, hp)
            if len(pending) >= DEPTH:
                pt_, php, pwts = pending.pop(0)
                emit_stage_b(pt_, php, pwts)
                tr_pending.append((pt_, php))
            if t > 0:
                # spread the previous tile's output projection across this
                # tile's iterations: keeps the PE continuously fed (HAM warm)
                # instead of a cold burst at each t boundary
                emit_proj_pair(t - 1, hp, pps_map[t - 1])
                if hp == HPAIRS - 1:
                    emit_fin(t - 1, pps_map.pop(t - 1))
            if len(tr_pending) > TR_EXTRA:
                emit_transpose(*tr_pending.pop(0))
            pending.append((t, hp, wts_))
        t_last = seq[-1][0]
        pps_last = alloc_pps()
        for ct in range(HPAIRS - DEPTH - TR_EXTRA):
            emit_proj_pair(t_last, ct, pps_last)
        while pending:
            pt_, php, pwts = pending.pop(0)
            emit_stage_b(pt_, php, pwts)
            tr_pending.append((pt_, php))
            trt, trp = tr_pending.pop(0)
            emit_transpose(trt, trp)
            emit_proj_pair(trt, trp, pps_last)
        while tr_pending:
            trt, trp = tr_pending.pop(0)
            emit_transpose(trt, trp)
            emit_proj_pair(trt, trp, pps_last)
        emit_fin(t_last, pps_last)

    _dedup_ldweights(nc)
    _split_excess_waits(nc)
    return nc


_NC_CACHE = None


def _get_nc() -> bass.Bass:
    global _NC_CACHE
    if _NC_CACHE is None:
        _NC_CACHE = build_nc()
    return _NC_CACHE


# ---------------------------------------------------------------------------
# Host side: shard, pre-transpose, cast; run SPMD; gather
# ---------------------------------------------------------------------------
def make_in_maps(x, qkv_w, qkv_b, proj_w, proj_b):
    x = np.asarray(x, np.float32)
    qkv_w = np.asarray(qkv_w, np.float32)
    qkv_b = np.asarray(qkv_b, np.float32)
    proj_w = np.asarray(proj_w, np.float32)
    proj_b = np.asarray(proj_b, np.float32)

    # fold the attention scale into the Q projection
    qkv_w = qkv_w.copy()
    qkv_b = qkv_b.copy()
    qkv_w[:C] *= SCALE
    qkv_b[:C] *= SCALE

    wv_np = np.ascontiguousarray(qkv_w[2 * C :].T).astype(BF16)

    def _ct_major(w):  # [c_out, c_in] -> [ct, p, ki*128]
        wt_ = w.T  # [c_in, c_out]
        return np.ascontiguousarray(
            wt_.reshape(KI, P, KI, P).transpose(2, 1, 0, 3).reshape(KI, P, C)
        ).astype(BF16)

    wq_np = _ct_major(qkv_w[:C])
    wk_np = _ct_major(qkv_w[C : 2 * C])
    pwt_np = np.ascontiguousarray(proj_w.T).astype(BF16)
    vb_np = np.ascontiguousarray(qkv_b[2 * C :])
    pb_np = proj_b

    slopes = _alibi_slopes(H)
    pp = np.arange(P, dtype=np.float32)

    # misc pack [P, 36] f32 (see device decl for the column map)
    misc_np = np.zeros((P, 36), np.float32)
    misc_np[:, 0:16] = qkv_b[: 2 * C].reshape(16, P).T
    misc_np[:, 16:32] = slopes[None, :] * (pp[:, None] - CBIAS)
    misc_np[:, 16] = 0.0
    for ji, j in enumerate((1, 2)):
        misc_np[:, 32 + ji] = slopes[0] * (j * P + pp - 320.0)
    misc_np[:, 34:36] = 1.0
    misc0_np = misc_np.copy()
    misc0_np[:, 34:36] = 0.0  # core 0: halo V rows 0-255 killed

    # per-head mask tiles [P(key), H, 3, P(query)]: {0,1} window/causal
    # pattern x per-(h, jblk) scalar exp(128*slope*(jblk-2)); head 0 plain.
    kk = pp[:, None]
    qq = pp[None, :]
    upper = (kk > qq).astype(np.float32)
    lower = (kk <= qq).astype(np.float32)
    ones = np.ones((P, P), np.float32)
    maskh_np = np.zeros((P, H, 3, P), np.float32)
    for h in range(H):
        c0 = math.exp(-256.0 * float(slopes[h]))
        c1 = math.exp(-128.0 * float(slopes[h]))
        if h == 0:
            maskh_np[:, h, 1] = ones
            maskh_np[:, h, 2] = lower
        else:
            if h >= 8:
                maskh_np[:, h, 0] = upper * c0
            maskh_np[:, h, 1] = ones * c1
            maskh_np[:, h, 2] = lower
    maskh_np = np.ascontiguousarray(maskh_np.reshape(P, H * 3 * P)).astype(BF16)

    in_maps = []
    for core in range(NCORES):
        b, c = divmod(core, NCHUNK)
        n0 = c * CH
        xh = np.zeros((ROWS, C), np.float32)
        lo = max(0, n0 - HALO)
        xh[HALO - (n0 - lo) :] = x[b, lo : n0 + CH]
        in_maps.append(
            {
                "xt": np.ascontiguousarray(xh.T).astype(BF16),
                "wv": wv_np,
                "wq": wq_np,
                "wk": wk_np,
                "pwt": pwt_np,
                "vb": vb_np,
                "pb": pb_np,
                "misc": misc0_np if c == 0 else misc_np,
                "maskh": maskh_np,
            }
        )
    return in_maps


def run(in_maps, trace=False, **kw):
    res = run_bass_kernel_spmd(
        _get_nc(), in_maps, core_ids=list(range(NCORES)), trace=trace, **kw
    )
    return res


def assemble(res):
    out = np.empty((B, N, C), np.float32)
    for core in range(NCORES):
        b, c = divmod(core, NCHUNK)
        out[b, c * CH : (c + 1) * CH] = res.results[core]["out"]
    return out


def kernel(x, qkv_w, qkv_b, proj_w, proj_b):
    in_maps = make_in_maps(x, qkv_w, qkv_b, proj_w, proj_b)
    res = run(in_maps)
    return assemble(res)

